# revision 23
# baseline (speedup 1.0000x reference)
"""DANet forward on 8 Trainium2 NeuronCores (Bass/Tile), pair-split scheme.

Core pair (s, s+4) handles sample s:
  - Core A (pid<4): conv5a -> feat1, full q/k/v, PAM attention for pam rows
    0..47 (queries 0..3071), conv51 rows 0..46, w8 partial (rows 0..46).
  - Core B (pid>=4): conv5c -> feat2, full CAM branch, conv52, w8(sc) full;
    then conv5a again (duplicated), attention tail rows 46..63, conv51 rows
    47..63, w8(sa-tail) added in.
  Host sums the two [19, 4096] partials per sample.

Layouts (per core / per sample):
  - Activations channel-major: [C=128 partitions, pixels].
  - Conv inputs zero-padded PAD = [128, 70, 66]: valid pixel (h, w) at
    [:, 3+h, 1+w].  Conv outputs IMG = [128, 66, 66]: valid (h, w) at
    [:, 1+h, 1+w]; borders hold garbage (never read).
  - BN folded into conv weights/bias on host.  PAM softmax unstabilized
    (max|score| ~60, fp32-safe); denominator via ones-row matmul.
  - PSUM tiles are all [128, 512] (one 2KB bank): S x3, av x2, den x2 =
    14KB, leaving headroom so accumulators double-buffer across tiles.
"""

import numpy as np
import sys

for p in ("/opt/trn_rl_repo",):
    if p not in sys.path:
        sys.path.insert(0, p)

import concourse.bass as bass
import concourse.tile as tile
from concourse import bacc, bass_isa, mybir
from concourse.bass_utils import run_bass_kernel_spmd
from concourse.masks import make_identity

F32 = mybir.dt.float32
BF16 = mybir.dt.bfloat16
AF = mybir.ActivationFunctionType
ALU = mybir.AluOpType
F32R = mybir.dt.float32r


def _mm(nc, out, lhsT, rhs, **kw):
    nc.tensor.matmul(out, lhsT.bitcast(F32R), rhs.bitcast(F32R), **kw)


H = W = 64
N = H * W              # 4096
CIN = 512
C = 128                # INTER
QK = 16
COUT = 19
NCHUNK = CIN // C      # 4
PW = 66                # padded row width
EPS = 1e-3

# A computes att/pam rows [0,48) and sa rows [0,47); B att rows [46,64),
# sa rows [47,64).  (Balance point measured in TimelineSim.)

# attention query tiles (q0, width); width >= 256 keeps fp32r at full rate
ATT_TILES_A = [(t * 512, 512) for t in range(6)]                  # rows 0..47
ATT_TILES_B = [(2944, 512), (3456, 384), (3840, 256)]             # rows 46..63
CONV_BLOCKS_A = [(0, 7), (7, 8), (15, 8), (23, 8), (31, 8), (39, 8)]   # sa rows 0..46
CONV_BLOCKS_B = [(47, 8), (55, 8), (63, 1)]                       # sa rows 47..63
Q_TILES_A = [(8 * t, 8) for t in range(6)]                        # feat rows for qT
Q_TILES_B = [(46, 8), (54, 8), (62, 2)]
OUT_COLS_A = 47 * 64
DEN_GPSIMD = 2         # every 2nd kc's denominator reduce runs on GPSIMD


def _ps3(t):
    return t.rearrange("p (a b) -> p a b", b=64)


def _emit_conv_from_x(nc, sb, pools, xTs, w_name, b_name, dst, post_block=None):
    """3x3 SAME conv from x (4 cin chunks): chunk-outer so compute starts as
    soon as chunk 0's DMA lands.  Two half-image passes of 4x 8-row PSUM
    blocks.  dst is IMG layout; only valid pixels written."""
    pool_S, pool_av, pool_den = pools
    w_sb = sb.tile([128, NCHUNK, 9, C], F32R, tag="convw")
    for c in range(NCHUNK):
        nc.scalar.dma_start(out=w_sb[:, c], in_=nc.input_aps[w_name][:, c])
    b_sb = sb.tile([C, 1], F32, tag="b12")
    nc.sync.dma_start(out=b_sb, in_=nc.input_aps[b_name][:])
    for half in range(2):
        blocks = [
            pool_S.tile([128, 512], F32, tag="S", name="cb0"),
            pool_S.tile([128, 512], F32, tag="S", name="cb1"),
            pool_av.tile([128, 512], F32, tag="av", name="cb2"),
            pool_den.tile([128, 512], F32, tag="den", name="cb3"),
        ]
        for c in range(NCHUNK):
            for k, (dy, dx) in enumerate(
                (dy, dx) for dy in range(3) for dx in range(3)
            ):
                for q in range(4):
                    h0 = half * 32 + q * 8
                    _mm(nc,
                        _ps3(blocks[q]),
                        w_sb[:, c, k, :],
                        xTs[c][:, 2 + h0 + dy : 2 + h0 + dy + 8, dx : dx + W],
                        start=(c == 0 and k == 0),
                        stop=(c == NCHUNK - 1 and k == 8),
                    )
        for q in range(4):
            h0 = half * 32 + q * 8
            nc.scalar.activation(
                out=dst[:, 1 + h0 : 9 + h0, 1:65],
                in_=_ps3(blocks[q]),
                func=AF.Relu,
                bias=b_sb,
                scale=1.0,
            )
            if post_block is not None:
                post_block(h0)


def _emit_conv3_rows(nc, pool_S, w_sb, b_sb, src_pad, dst, blocks, relu=True):
    """3x3 SAME conv on row blocks: src_pad is PAD [128,70,66] (valid (h,w) at
    [:,3+h,1+w]); dst is compact [128, 64, 64] (row h at dst[:, h, :])."""
    for h0, nr in blocks:
        ps = _ps3(pool_S.tile([128, 512], F32, tag="S", name="convps"))
        first = True
        for k, (dy, dx) in enumerate((dy, dx) for dy in range(3) for dx in range(3)):
            _mm(nc,
                ps[:, :nr, :],
                w_sb[:, 0, k, :],
                src_pad[:, 2 + h0 + dy : 2 + h0 + dy + nr, dx : dx + W],
                start=first,
                stop=(k == 8),
            )
            first = False
        nc.scalar.activation(
            out=dst[:, h0 : h0 + nr, :],
            in_=ps[:, :nr, :],
            func=AF.Relu if relu else AF.Identity,
            bias=b_sb,
            scale=1.0,
        )


def _emit_qkv(nc, sb, pool_S, feat1T, q_tiles):
    """qT (only q_tiles), kT (full), v_sb (full); qT/kT reuse dead xT slots."""
    qT = sb.tile([16, N], F32R, tag="xT0")
    kT = sb.tile([16, N], F32R, tag="xT1")
    for dst, w_name, b_name, tiles in (
        (qT, "wq", "bq", q_tiles),
        (kT, "wk", "bk", [(8 * t, 8) for t in range(8)]),
    ):
        w_sb = sb.tile([128, QK], F32R, tag=w_name)
        nc.sync.dma_start(out=w_sb, in_=nc.input_aps[w_name][:])
        b_sb = sb.tile([QK, 1], F32, tag=b_name)
        nc.sync.dma_start(out=b_sb, in_=nc.input_aps[b_name][:])
        for r0, nr in tiles:
            ps = _ps3(pool_S.tile([128, 512], F32, tag="S", name="qkps"))
            _mm(nc,
                ps[:QK, :nr, :],
                w_sb,
                feat1T[:, 1 + r0 : 1 + r0 + nr, 1:65],
                start=True,
                stop=True,
            )
            nc.scalar.activation(
                out=qT_slice(dst, r0, nr),
                in_=ps[:QK, :nr, :],
                func=AF.Identity,
                bias=b_sb,
                scale=1.0,
            )

    return qT, kT


def _emit_v(nc, sb, pools, feat1T):
    """v in av-lhsT layout [pix-in-chunk, kc, ch]; bv folded in at the end.
    One 8-row DVE copy feeds four contiguous-slice matmuls."""
    pool_S, pool_av, pool_den = pools
    wv_sb = sb.tile([128, C], F32R, tag="wv")
    nc.sync.dma_start(out=wv_sb, in_=nc.input_aps["wv"][:])
    v_sb = sb.tile([128, 32, C], F32R, tag="vf")
    for h0 in range(0, 64, 8):
        f1b = sb.tile([128, 512], F32R, tag="f12c", bufs=2)
        nc.vector.tensor_copy(_ps3(f1b), feat1T[:, 1 + h0 : 9 + h0, 1:65])
        for r in range(4):
            i = h0 // 2 + r
            ps = pool_S.tile([128, 512], F32, tag="S", name="vps")
            _mm(nc, ps[:, :C], f1b[:, r * C : (r + 1) * C], wv_sb,
                start=True, stop=True)
            nc.scalar.copy(v_sb[:, i, :], ps[:, :C])
    return v_sb


def qT_slice(qT, r0, nr):
    return qT[:, r0 * 64 : (r0 + nr) * 64]


def _emit_attention(nc, sb, pools, feat1T, pamT, qT, kT, v_sb, att_tiles,
                    thunks_by_tile=None, den_mod=DEN_GPSIMD):
    """PAM attention for the given query tiles; writes pam rows (PAD layout)
    = gamma*softmax(qk)v + feat1 + gamma*bv.  thunks_by_tile[ti] holds
    deferred single-matmul emitters (conv51/w8 work) that become legal once
    tile ti is finished; they are drained one per GPSIMD-denominator step to
    fill PE idle while ACT paces the exp."""
    pool_S, pool_av, pool_den = pools
    ones_col = sb.tile([128, 1], F32R, tag="ones_col")
    nc.vector.memset(ones_col.bitcast(F32), 1.0)
    g07 = sb.tile([1, 128], F32, tag="g07")
    nc.sync.dma_start(out=g07, in_=nc.input_aps["gpam_row"][:])
    bv07 = sb.tile([C, 1], F32, tag="bv07")
    nc.sync.dma_start(out=bv07, in_=nc.input_aps["bv07"][:])

    # software-pipelined emission over (tile, kc): the S matmul for step i+1
    # is emitted before av/den of step i so PE never idles behind ACT's exp.
    steps = [(ti, kc) for ti in range(len(att_tiles)) for kc in range(32)]

    def emit_S(ti, kc):
        q0, tw = att_tiles[ti]
        S = pool_S.tile([128, 512], F32, tag="S", name="Sps")
        _mm(nc,
            S[:, :tw],
            kT[:, kc * 128 : (kc + 1) * 128],
            qT[:, q0 : q0 + tw],
            start=True,
            stop=True,
        )
        return S

    avs = {}
    ready = []
    pending = None
    S_cur = emit_S(*steps[0])
    for i, (ti, kc) in enumerate(steps):
        q0, tw = att_tiles[ti]
        if kc == 0:
            avs[ti] = (
                pool_av.tile([128, 512], F32, tag="av", name="avps"),
                pool_den.tile([128, 512], F32, tag="den", name="denps"),
                sb.tile([1, 512], F32, tag="den_acc", bufs=2, name="den_acc"),
            )
        av, den, den_acc = avs[ti]
        expS = sb.tile([128, 512], F32R, tag="expS", bufs=3)
        nc.scalar.activation(out=expS[:, :tw], in_=S_cur[:, :tw], func=AF.Exp)
        if i + 1 < len(steps):
            S_nxt = emit_S(*steps[i + 1])
        _mm(nc,
            av[:, :tw], v_sb[:, kc, :], expS[:, :tw],
            start=(kc == 0), stop=(kc == 31),
        )
        # denominator (colsum of expS): mostly a PE ones-matmul accumulating
        # in PSUM; every DEN_GPSIMD'th kc offloads to GPSIMD (idle engine)
        # with a cheap [1,512] DVE accumulate.
        if kc % den_mod == 0:
            dar = sb.tile([128, 512], F32, tag="dar", bufs=2)
            nc.gpsimd.partition_all_reduce(
                dar[:, :tw], expS[:, :tw].bitcast(F32), channels=128,
                reduce_op=bass_isa.ReduceOp.add,
            )
            if kc == 0:
                nc.vector.tensor_copy(den_acc[:, :tw], dar[:1, :tw])
            else:
                nc.vector.tensor_add(den_acc[:, :tw], den_acc[:, :tw], dar[:1, :tw])
        else:
            _mm(nc,
                den[:1, :tw], ones_col, expS[:, :tw],
                start=(kc == 1), stop=(kc == 31),
            )
        if kc % den_mod == 0 and kc > 0 and ready:
            ready.pop(0)()
        if pending is not None:
            _finish_tile(nc, sb, pool_S, feat1T, pamT, g07, bv07, *pending)
            pending = None
        if kc == 31:
            pending = (q0, tw, av, den, den_acc)
            del avs[ti]
            if thunks_by_tile and ti in thunks_by_tile:
                ready.extend(thunks_by_tile[ti])
        if i + 1 < len(steps):
            S_cur = S_nxt
    if pending is not None:
        _finish_tile(nc, sb, pool_S, feat1T, pamT, g07, bv07, *pending)
    for fn in ready:
        fn()


def _finish_tile(nc, sb, pool_S, feat1T, pamT, g07, bv07, q0, tw,
                 av, den, den_acc):
    """normalize + residual + bias for one finished attention tile."""
    nc.vector.tensor_add(den_acc[:, :tw], den_acc[:, :tw], den[:1, :tw])
    # rb = 0.7/denom broadcast to 128 partitions via K=1 matmul
    rb_row = sb.tile([1, 512], F32, tag="rb_row", bufs=1)
    nc.vector.reciprocal(rb_row[:, :tw], den_acc[:, :tw])
    rb = pool_S.tile([128, 512], F32, tag="S")
    nc.tensor.matmul(rb[:, :tw], g07, rb_row[:, :tw], start=True, stop=True)
    rb_bc = sb.tile([128, 512], F32, tag="rb_bc", bufs=1)
    nc.scalar.copy(rb_bc[:, :tw], rb[:, :tw])
    r0, nr = q0 // 64, tw // 64
    psl = pamT[:, 3 + r0 : 3 + r0 + nr, 1:65]
    nc.vector.tensor_mul(
        psl,
        _ps3(av)[:, :nr, :],
        rb_bc[:, :tw].rearrange("p (a b) -> p a b", b=64),
    )
    nc.vector.tensor_add(psl, psl, feat1T[:, 1 + r0 : 1 + r0 + nr, 1:65])
    nc.scalar.activation(out=psl, in_=psl, func=AF.Identity, bias=bv07, scale=1.0)


def _tail_thunks(nc, sb, pool_S, w51_sb, b3_sb, w8_sb, pamT, saT, outT,
                blocks, n_att_tiles, row2tile, accumulate):
    """Build thunks_by_tile for conv51 + w8 row blocks."""
    thunks = {}

    def add(ti, fn):
        thunks.setdefault(min(ti, n_att_tiles - 1), []).append(fn)

    for h0, nr in blocks:
        ti = row2tile(h0 + nr)
        state = {}

        def mk_conv(k, dy, dx, h0=h0, nr=nr, state=state):
            def fn():
                if "ps" not in state:
                    state["ps"] = _ps3(
                        pool_S.tile([128, 512], F32, tag="S", name="tailps")
                    )
                _mm(nc,
                    state["ps"][:, :nr, :],
                    w51_sb[:, 0, k, :],
                    pamT[:, 2 + h0 + dy : 2 + h0 + dy + nr, dx : dx + W],
                    start=(k == 0),
                    stop=(k == 8),
                )
                if k == 8:
                    nc.scalar.activation(
                        out=saT[:, h0 : h0 + nr, :],
                        in_=state["ps"][:, :nr, :],
                        func=AF.Relu,
                        bias=b3_sb,
                        scale=1.0,
                    )
            return fn

        for k, (dy, dx) in enumerate((dy, dx) for dy in range(3) for dx in range(3)):
            add(ti, mk_conv(k, dy, dx))

        def mk_w8(h0=h0, nr=nr):
            def fn():
                ps = pool_S.tile([128, 512], F32, tag="S", name="w8ps")
                _mm(nc,
                    _ps3(ps)[:COUT, :nr, :], w8_sb, saT[:, h0 : h0 + nr, :],
                    start=True, stop=True,
                )
                cols = slice(h0 * 64, (h0 + nr) * 64)
                if accumulate:
                    nc.vector.tensor_add(outT[:, cols], ps[:COUT, : nr * 64],
                                         outT[:, cols])
                else:
                    nc.scalar.copy(outT[:, cols], ps[:COUT, : nr * 64])
                nc.sync.dma_start(out=nc.out_d[:, cols], in_=outT[:, cols])
            return fn

        add(ti, mk_w8())
    return thunks


def _emit_f2n(nc, sb, pools, feat2T):
    """f2n = per-chunk transposed feat2 ([pix, ch]); one 8-row DVE copy
    feeds four transposes."""
    pool_S, pool_av, pool_den = pools
    identity = sb.tile([128, 128], F32, tag="identity")
    make_identity(nc, identity)
    f2n = sb.tile([128, 32, C], F32R, tag="vf")
    for h0 in range(0, 64, 8):
        f2b = sb.tile([128, 512], F32, tag="f12c", bufs=2)
        nc.vector.tensor_copy(_ps3(f2b), feat2T[:, 1 + h0 : 9 + h0, 1:65])
        for r in range(4):
            i = h0 // 2 + r
            ps = pool_S.tile([128, 512], F32, tag="S", name="tps")
            nc.tensor.transpose(ps[:, :C], f2b[:, r * C : (r + 1) * C], identity)
            nc.vector.tensor_copy(f2n[:, i, :], ps[:, :C])
    return f2n, identity


def _emit_cam(nc, sb, pool_S, pool_av, feat2T, camT, f2n, identity):
    """Channel attention; writes cam into camT (PAD layout)."""
    gcam = sb.tile([128, 1], F32, tag="gcam")
    nc.sync.dma_start(out=gcam, in_=nc.input_aps["gcam_col"][:])
    ef = pool_av.tile([128, 512], F32, tag="av")
    for i in range(32):
        _mm(nc,
            ef[:, :C], f2n[:, i, :], f2n[:, i, :],
            start=(i == 0), stop=(i == 31),
        )
    # attc = softmax(rowmax - energy) == softmax(-energy), stabilized by rowmin
    emin = sb.tile([128, 1], F32, tag="emin")
    nc.vector.tensor_reduce(
        out=emin, in_=ef[:, :C], axis=mybir.AxisListType.X, op=ALU.min
    )
    attc = sb.tile([128, C], F32, tag="attc")
    nc.scalar.activation(
        out=attc, in_=ef[:, :C], func=AF.Exp, bias=emin, scale=-1.0
    )
    esum = sb.tile([128, 1], F32, tag="esum")
    nc.vector.reduce_sum(out=esum, in_=attc, axis=mybir.AxisListType.X)
    erec = sb.tile([128, 1], F32, tag="erec")
    nc.vector.reciprocal(erec, esum)
    attcn = sb.tile([128, C], F32, tag="attcn")
    nc.vector.tensor_scalar_mul(attcn, attc, erec)
    ptf = pool_S.tile([128, 512], F32, tag="S")
    nc.tensor.transpose(ptf[:, :C], attcn, identity)
    attcT = sb.tile([128, C], F32R, tag="attcT")
    nc.vector.tensor_copy(attcT, ptf[:, :C])

    for t in range(8):
        ps = _ps3(pool_S.tile([128, 512], F32, tag="S", name="camps"))
        _mm(nc,
            ps[:, :8, :], attcT, feat2T[:, 1 + 8 * t : 1 + 8 * t + 8, 1:65],
            start=True, stop=True,
        )
        nc.vector.scalar_tensor_tensor(
            out=camT[:, 3 + 8 * t : 3 + 8 * t + 8, 1:65],
            in0=ps[:, :8, :],
            scalar=gcam,
            in1=feat2T[:, 1 + 8 * t : 1 + 8 * t + 8, 1:65],
            op0=ALU.mult,
            op1=ALU.add,
        )


def _emit_w8(nc, sb, pool_S, w8_sb, src, blocks, outT, accumulate):
    """1x1 conv w8 over row blocks of compact src; copy or add into outT."""
    for h0, nr in blocks:
        ps = pool_S.tile([128, 512], F32, tag="S")
        _mm(nc,
            _ps3(ps)[:COUT, :nr, :], w8_sb, src[:, h0 : h0 + nr, :],
            start=True, stop=True,
        )
        dst = outT[:, h0 * 64 : (h0 + nr) * 64]
        if accumulate:
            nc.vector.tensor_add(dst, ps[:COUT, : nr * 64], dst)
        else:
            nc.scalar.copy(dst, ps[:COUT, : nr * 64])


def build_program_pair():
    nc = bacc.Bacc("TRN2", target_bir_lowering=False, debug=False)
    nc.input_aps = {}
    R_INPUTS = {"xT", "w5at", "wq", "wk", "wv", "w5ct", "w51t", "w52t", "w8"}

    def din(name, shape):
        dt = F32R if name in R_INPUTS else F32
        h = nc.dram_tensor(name, shape, dt, kind="ExternalInput")
        nc.input_aps[name] = h[:]
        return h

    din("xT", [NCHUNK * C, 70, PW])
    din("w5at", [C, NCHUNK, 9, C])
    din("b1", [C, 1])
    din("wq", [C, QK])
    din("bq", [QK, 1])
    din("wk", [C, QK])
    din("bk", [QK, 1])
    din("wv", [C, C])
    din("bv07", [C, 1])
    din("w5ct", [C, NCHUNK, 9, C])
    din("b2", [C, 1])
    din("w51t", [C, 1, 9, C])
    din("b3", [C, 1])
    din("w52t", [C, 1, 9, C])
    din("b4", [C, 1])
    din("w8", [C, COUT])
    din("gpam_row", [1, C])
    din("gcam_col", [C, 1])
    nc.out_d = nc.dram_tensor("out", [COUT, N], F32, kind="ExternalOutput")

    with tile.TileContext(nc) as tc:
        with (
            tc.tile_pool(name="sb", bufs=1) as sb,
            tc.tile_pool(name="psS", bufs=3, space="PSUM") as pool_S,
            tc.tile_pool(name="psav", bufs=2, space="PSUM") as pool_av,
            tc.tile_pool(name="psden", bufs=2, space="PSUM") as pool_den,
        ):
            pools = (pool_S, pool_av, pool_den)
            pid = nc.partition_id()
            featT = sb.tile([128, PW, PW], F32R, tag="featT")
            bigpad = sb.tile([128, 70, PW], F32R, tag="bigpad")

            with tc.If(pid < 4) as cmp:
                _emit_pam_core(nc, sb, pools, featT, bigpad)
            with cmp.Else():
                _emit_cam_core(nc, sb, pools, featT, bigpad)

    nc.finalize()
    return nc


def _load_xT(nc, sb):
    """xT chunk tiles, allocated and DMA'd inside the branch so their slots
    (tags xT0/xT1, reused later by qT/kT) have a branch-local lifecycle."""
    xTs = []
    for c in range(NCHUNK):
        xc = sb.tile([128, 70, PW], F32R, tag=f"xT{c}", name=f"xT{c}")
        src = nc.input_aps["xT"][c * C : (c + 1) * C]
        if c == 0:
            nc.sync.dma_start(out=xc[:, :14], in_=src[:, :14])
            nc.sync.dma_start(out=xc[:, 14:37], in_=src[:, 14:37])
        else:
            nc.sync.dma_start(out=xc[:, :37], in_=src[:, :37])
        nc.sync.dma_start(out=xc[:, 37:], in_=src[:, 37:])
        xTs.append(xc)
    return xTs


def _emit_pam_core(nc, sb, pools, featT, bigpad):
    pool_S, pool_av, pool_den = pools
    xTs = _load_xT(nc, sb)
    _emit_conv_from_x(nc, sb, pools, xTs, "w5at", "b1", featT)
    nc.vector.memset(bigpad.bitcast(F32), 0.0)
    qT, kT = _emit_qkv(nc, sb, pool_S, featT, Q_TILES_A)
    v_sb = _emit_v(nc, sb, pools, featT)
    w51_sb = sb.tile([128, 1, 9, C], F32R, tag="convw2")
    nc.sync.dma_start(out=w51_sb, in_=nc.input_aps["w51t"][:])
    b3_sb = sb.tile([C, 1], F32, tag="b3")
    nc.sync.dma_start(out=b3_sb, in_=nc.input_aps["b3"][:])
    saT = sb.tile([128, 64, W], F32R, tag="saT")
    w8_sb = sb.tile([128, COUT], F32R, tag="w8")
    nc.sync.dma_start(out=w8_sb, in_=nc.input_aps["w8"][:])
    outT = sb.tile([COUT, N], F32, tag="outT")
    nc.vector.memset(outT[:, OUT_COLS_A:], 0.0)
    thunks = _tail_thunks(nc, sb, pool_S, w51_sb, b3_sb, w8_sb, bigpad, saT,
                          outT, CONV_BLOCKS_A, len(ATT_TILES_A),
                          lambda rend: rend // 8, accumulate=False)
    nc.sync.dma_start(out=nc.out_d[:, OUT_COLS_A:], in_=outT[:, OUT_COLS_A:])
    _emit_attention(nc, sb, pools, featT, bigpad, qT, kT, v_sb, ATT_TILES_A,
                    thunks, den_mod=3)


def _emit_cam_core(nc, sb, pools, featT, bigpad):
    pool_S, pool_av, pool_den = pools
    xTs = _load_xT(nc, sb)
    _emit_conv_from_x(nc, sb, pools, xTs, "w5ct", "b2", featT)
    nc.vector.memset(bigpad.bitcast(F32), 0.0)
    f2n, identity = _emit_f2n(nc, sb, pools, featT)
    _emit_cam(nc, sb, pool_S, pool_av, featT, bigpad, f2n, identity)
    w52_sb = sb.tile([128, 1, 9, C], F32R, tag="convw2")
    nc.sync.dma_start(out=w52_sb, in_=nc.input_aps["w52t"][:])
    b4_sb = sb.tile([C, 1], F32, tag="b4")
    nc.sync.dma_start(out=b4_sb, in_=nc.input_aps["b4"][:])
    saT = sb.tile([128, 64, W], F32R, tag="saT")
    _emit_conv3_rows(nc, pool_S, w52_sb, b4_sb, bigpad, saT,
                     [(8 * t, 8) for t in range(8)])
    w8_sb = sb.tile([128, COUT], F32R, tag="w8")
    nc.sync.dma_start(out=w8_sb, in_=nc.input_aps["w8"][:])
    outT = sb.tile([COUT, N], F32, tag="outT")
    _emit_w8(nc, sb, pool_S, w8_sb, saT, [(8 * t, 8) for t in range(8)], outT,
             accumulate=False)
    nc.sync.dma_start(out=nc.out_d[:, : 57 * 64], in_=outT[:, : 57 * 64])

    # --- PAM tail ---
    _emit_conv_from_x(nc, sb, pools, xTs, "w5at", "b1", featT)
    nc.vector.memset(bigpad.bitcast(F32), 0.0)
    qT, kT = _emit_qkv(nc, sb, pool_S, featT, Q_TILES_B)
    v_sb = _emit_v(nc, sb, pools, featT)
    w51_sb = sb.tile([128, 1, 9, C], F32R, tag="convw2")
    nc.sync.dma_start(out=w51_sb, in_=nc.input_aps["w51t"][:])
    b3_sb = sb.tile([C, 1], F32, tag="b3")
    nc.sync.dma_start(out=b3_sb, in_=nc.input_aps["b3"][:])
    row0_b = ATT_TILES_B[0][0] // 64

    def row2tile_b(rend):
        for ti, (q0, tw) in enumerate(ATT_TILES_B):
            if rend <= (q0 + tw) // 64:
                return ti
        return len(ATT_TILES_B) - 1

    thunks = _tail_thunks(nc, sb, pool_S, w51_sb, b3_sb, w8_sb, bigpad, saT,
                          outT, CONV_BLOCKS_B, len(ATT_TILES_B), row2tile_b,
                          accumulate=True)
    _emit_attention(nc, sb, pools, featT, bigpad, qT, kT, v_sb, ATT_TILES_B,
                    thunks, den_mod=2)


# ---------------- host side ----------------

def _fold_bn(w, g, b, m, v):
    s = g / np.sqrt(v + EPS)
    return (w * s).astype(np.float32), (b - m * s).astype(np.float32)


def _conv_w_layout(w):
    # [3,3,cin,cout] -> [128, cin//128, 9, cout]
    cin = w.shape[2]
    nch = cin // C
    return np.ascontiguousarray(
        w.reshape(9, nch, C, w.shape[3]).transpose(2, 1, 0, 3)
    ).astype(np.float32)


def _pad_xT(x):
    # x [H, W, 512] -> [512, 70, 66] zero-padded channel-major
    xp = np.zeros((CIN, 70, PW), np.float32)
    xp[:, 3 : 3 + H, 1 : 1 + W] = x.transpose(2, 0, 1)
    return xp


_CACHED = {}


def prepare(inputs):
    """Returns (nc, in_maps) for the 8-core SPMD launch."""
    inputs = {k: np.asarray(v) for k, v in inputs.items()}
    x = inputs["x"]
    B = x.shape[0]

    w5a, b1 = _fold_bn(inputs["w5a"], inputs["bn1_g"], inputs["bn1_b"],
                       inputs["bn1_m"], inputs["bn1_v"])
    w5c, b2 = _fold_bn(inputs["w5c"], inputs["bn2_g"], inputs["bn2_b"],
                       inputs["bn2_m"], inputs["bn2_v"])
    w51, b3 = _fold_bn(inputs["w51"], inputs["bn3_g"], inputs["bn3_b"],
                       inputs["bn3_m"], inputs["bn3_v"])
    w52, b4 = _fold_bn(inputs["w52"], inputs["bn4_g"], inputs["bn4_b"],
                       inputs["bn4_m"], inputs["bn4_v"])
    gp = float(inputs["gamma_pam"])
    gc = float(inputs["gamma_cam"])

    common = dict(
        w5at=_conv_w_layout(w5a), b1=b1.reshape(C, 1),
        wq=np.ascontiguousarray(inputs["wq"][0, 0]), bq=inputs["bq"].reshape(QK, 1),
        wk=np.ascontiguousarray(inputs["wk"][0, 0]), bk=inputs["bk"].reshape(QK, 1),
        wv=np.ascontiguousarray(inputs["wv"][0, 0]),
        bv07=(gp * inputs["bv"]).reshape(C, 1).astype(np.float32),
        w5ct=_conv_w_layout(w5c), b2=b2.reshape(C, 1),
        w51t=_conv_w_layout(w51), b3=b3.reshape(C, 1),
        w52t=_conv_w_layout(w52), b4=b4.reshape(C, 1),
        w8=np.ascontiguousarray(inputs["w8"][0, 0]),
        gpam_row=np.full((1, C), gp, np.float32),
        gcam_col=np.full((C, 1), gc, np.float32),
    )

    if "nc_pair" not in _CACHED:
        _CACHED["nc_pair"] = build_program_pair()
    nc = _CACHED["nc_pair"]

    in_maps = []
    for core in range(8):
        s = core % B
        in_maps.append({"xT": _pad_xT(x[s]), **common})
    return nc, in_maps


def kernel(**inputs):
    B = np.asarray(inputs["x"]).shape[0]
    nc, in_maps = prepare(inputs)
    res = run_bass_kernel_spmd(nc, in_maps, core_ids=list(range(8)))
    _CACHED["last_result"] = res
    out = np.zeros((B, H, W, COUT), np.float32)
    for s in range(B):
        o = res.results[s]["out"] + res.results[s + 4]["out"]  # [19, 4096]
        out[s] = o.T.reshape(H, W, COUT)
    return out
